# revision 26
# baseline (speedup 1.0000x reference)
"""Cached multi-head attention, sharded over heads across 8 TRN2 NeuronCores.

Per-core work (2 of 16 heads, all 8 batches), fp8-accelerated:
  - K/V caches stored in fp8e3 (E3M4), halving cache DMA; V quad-packed so
    DMA runs are 512B. The 128 new keys/values are read directly from the
    fp16 qkvT / vnew tiles as the chunk-31 matmul stationaries (never
    quantized to fp8).
  - Scores / attn@V matmuls: fp8e3 stationary x fp16 moving (mixed dtype,
    bf16-rate; ~1.3% rms noise on cache reads only).
  - QKV projection: q columns fp16; k/v columns via fp8e4 DoubleRow
    (256-deep contraction, 2x PE rate; errors land only on the new keys).
  - exp on ACT; causal mask on the last key block; softmax denominator via
    DVE log2 fold + ones-stationary matmul; normalization by reciprocal
    partition-broadcast.
  - PSUM: 3 rotating 2-bank score tiles; one persistent bank packs both
    in-flight attn@V accumulators plus the denominators (subtile slices);
    one persistent bank double-buffers the output projection.
  - PSUM->SBUF copies run on the (otherwise idle) GpSimd engine.
  - Output projection in 256-token blocks -> one [128,16,256] staging tile
    and a single DMA per block; host sums the 8 partials + bias.
"""

import numpy as np

import concourse.bacc as bacc
import concourse.mybir as mybir
import concourse.tile as tile
from concourse.bass_utils import run_bass_kernel_spmd

B, Q, D = 8, 128, 2048
H, HD = 16, 128
CACHE = 3968
K = CACHE + Q          # 4096
NCORES = 8
HPC = H // NCORES      # heads per core
NKC = K // 128         # 32 key chunks (31 cache + 1 new)
NQ4 = 8                # quads of 512 keys for the packed V cache
TOK = B * Q            # 1024 tokens
SCALE = 1.0 / float(np.sqrt(HD))

F16 = mybir.dt.float16
F32 = mybir.dt.float32
F8E3 = mybir.dt.float8e3
F8E4 = mybir.dt.float8e4
DR = mybir.MatmulPerfMode.DoubleRow

_STATE = {}


def build_nc(reps=1):
    nc = bacc.Bacc("TRN2", target_bir_lowering=False, debug=False)

    xt_d = nc.dram_tensor("xt", [D, TOK], F16, kind="ExternalInput")
    xt8_d = nc.dram_tensor("xt8", [128, 8, 2, TOK], F8E4, kind="ExternalInput")
    wqq_d = nc.dram_tensor("wqq", [128, 16, 256], F16, kind="ExternalInput")
    wq8_d = nc.dram_tensor("wq8", [128, 8, 2, 512], F8E4, kind="ExternalInput")
    kt_d = nc.dram_tensor("kt8", [HPC, B, HD, CACHE], F8E3, kind="ExternalInput")
    vp_d = nc.dram_tensor("vp8", [HPC, B, NQ4, 128, 512], F8E3, kind="ExternalInput")
    wp_d = nc.dram_tensor("wp", [HPC * HD, D], F16, kind="ExternalInput")
    out_d = nc.dram_tensor("out", [D, TOK], F16, kind="ExternalOutput")

    with tile.TileContext(nc) as tc:
        with (
            tc.tile_pool(name="const", bufs=1) as cpool,
            tc.tile_pool(name="xw", bufs=1) as xwpool,
            tc.tile_pool(name="qkv", bufs=1) as qkvpool,
            tc.tile_pool(name="vnew", bufs=1) as vnewpool,
            tc.tile_pool(name="attn", bufs=1) as attnpool,
            tc.tile_pool(name="kt", bufs=6) as ktpool,
            tc.tile_pool(name="v", bufs=6) as vpool,
            tc.tile_pool(name="p", bufs=2) as ppool,
            tc.tile_pool(name="fold", bufs=2) as foldpool,
            tc.tile_pool(name="small", bufs=2) as smallpool,
            tc.tile_pool(name="ostage", bufs=4) as opool,
            tc.tile_pool(name="ps_s", bufs=2, space="PSUM") as psum_s,
            tc.tile_pool(name="ps_q", bufs=2, space="PSUM") as psum_q,
            tc.tile_pool(name="ps_acc", bufs=1, space="PSUM") as psum_a,
            tc.tile_pool(name="ps_proj", bufs=1, space="PSUM") as psum_p,
        ):
            # constants
            ones_col = cpool.tile([128, 1], F16)       # denominator stationary
            nc.vector.memset(ones_col[:], 1.0)
            ones_full = cpool.tile([128, 128], F16)
            nc.vector.memset(ones_full[:], 1.0)
            # causal mask for the last key block: keep (p=key j', free=query i)
            # where i >= j'  -> iota = i - j' >= 0
            zeros16 = cpool.tile([128, 128], F16)
            nc.vector.memset(zeros16[:], 0.0)
            # masktri[p=i, col=j'] = -60000 where j' > i (strict upper):
            # its transpose, accumulated into the chunk-31 score PSUM via a
            # constant matmul against the identity, applies the causal mask
            masktri = cpool.tile([128, 128], F16)
            nc.gpsimd.affine_select(
                masktri[:], zeros16[:], pattern=[[-1, 128]],
                compare_op=mybir.AluOpType.is_ge, fill=-60000.0,
                base=0, channel_multiplier=1,
            )
            ident = cpool.tile([128, 128], F16)
            nc.gpsimd.affine_select(
                ident[:], ones_full[:], pattern=[[1, 128]],
                compare_op=mybir.AluOpType.is_equal, fill=0.0,
                base=0, channel_multiplier=-1,
            )

            for _rep in range(reps):
                # one persistent bank: attn@V accumulators for both in-flight
                # pairs ([:,0:128],[:,128:256]) + denominators on partition 0
                # ([0:1,256:384],[0:1,384:512])
                ps_acc = psum_a.tile([128, 512], F32, tag="acc", name="ps_acc")
                # one persistent bank double-buffering the projection
                ps_proj = psum_p.tile([128, 512], F32, tag="proj", name="ps_proj")

                xt8_half = [None, None]
                xt_half = [None, None]

                def load_xt8(t):
                    xt8_half[t] = xwpool.tile([128, 8, 2, 512], F8E4,
                                              tag="xt8", name=f"xt8_{t}")
                    nc.sync.dma_start(
                        xt8_half[t][:], xt8_d.ap()[:, :, :, t * 512:(t + 1) * 512]
                    )

                def load_xt(t, s):
                    if xt_half[t] is None:
                        xt_half[t] = xwpool.tile([128, D // 128, 512], F16,
                                                 tag="xt", name=f"xt{t}")
                    nc.sync.dma_start(
                        xt_half[t][:, :, s * 256:(s + 1) * 256],
                        xt_d.ap().rearrange("(t p) n -> p t n", p=128)
                        [:, :, t * 512 + s * 256:t * 512 + (s + 1) * 256],
                    )

                wq8_sb = xwpool.tile([128, 8, 2, 512], F8E4, tag="wq8", name="wq8_sb")
                wqq_sb = xwpool.tile([128, 16, 256], F16, tag="wqq", name="wqq_sb")

                nc.sync.dma_start(wqq_sb[:], wqq_d.ap())
                load_xt(0, 0)

                # qkvT oc layout: [q0, q1, k0, k1, v0, v1] each [128, 512]
                qkvT = [
                    qkvpool.tile([128, 6, 512], F16, tag=f"qkvT{t}", name=f"qkvT{t}")
                    for t in range(2)
                ]
                vnew_sb = vnewpool.tile([128, HPC, B, HD], F16, tag="vnew",
                                        name="vnew_sb")

                def kv_dr(t, cb):
                    # k/v projection columns via fp8e4 DoubleRow (256-contraction)
                    ps = psum_q.tile([128, 512], F32, tag="ps_q", name="ps_kv")
                    for j in range(8):
                        nc.tensor.matmul(
                            ps[:],
                            wq8_sb[:, j, :, cb * 128:(cb + 1) * 128],
                            xt8_half[t][:, j, :, :],
                            start=(j == 0), stop=(j == 7),
                            perf_mode=DR,
                        )
                    nc.scalar.copy(qkvT[t][:, 2 + cb, :], ps[:])

                def q_part(t, hh, s):
                    # q columns, fp16, one 256-token slab
                    ps = psum_q.tile([128, 512], F32, tag="ps_q", name="ps_q")[:, 0:256]
                    for dt_ in range(D // 128):
                        nc.tensor.matmul(
                            ps[:],
                            wqq_sb[:, dt_, hh * 128:(hh + 1) * 128],
                            xt_half[t][:, dt_, s * 256:(s + 1) * 256],
                            start=(dt_ == 0), stop=(dt_ == D // 128 - 1),
                        )
                    nc.scalar.copy(
                        qkvT[t][:, hh, s * 256:(s + 1) * 256], ps[:]
                    )

                def vnew_transposes(t, hh):
                    # v_new natural layout via PE transpose for this half's batches
                    for bb in range(4):
                        b = 4 * t + bb
                        ps_t = psum_q.tile([128, 512], F32, tag="ps_q",
                                           name="ps_vt").bitcast(mybir.dt.float16)[:, 0:128]
                        nc.tensor.transpose(
                            ps_t[:], qkvT[t][:, 4 + hh, bb * 128:(bb + 1) * 128],
                            ident[:],
                        )
                        nc.vector.tensor_copy(vnew_sb[:, hh, b, :], ps_t[:])

                attn_sb = attnpool.tile([128, HPC, TOK], F16, tag="attn",
                                        name="attn_sb")
                wp_sb = xwpool.tile([128, HPC, D], F16, tag="wp", name="wp_sb")
                out_r = out_d.ap().rearrange("(cc p) n -> p cc n", p=128)

                o_bigs = {}

                def proj_block(tb):
                    # partial final^T for 256 tokens (batches 2tb, 2tb+1)
                    o_big = opool.tile([128, 16, 256], F16)
                    o_bigs[tb] = o_big
                    for cc in range(D // 128):
                        psl = ps_proj[:, (cc % 2) * 256:(cc % 2) * 256 + 256]
                        for ht in range(HPC):
                            nc.tensor.matmul(
                                psl,
                                wp_sb[:, ht, cc * 128:(cc + 1) * 128],
                                attn_sb[:, ht, tb * 256:(tb + 1) * 256],
                                start=(ht == 0), stop=(ht == HPC - 1),
                            )
                        if cc % 8 == 1:
                            nc.scalar.copy(o_big[:, cc, :], psl)
                        else:
                            nc.vector.tensor_copy(o_big[:, cc, :], psl)
                def proj_store(tb):
                    # emitted one batch after proj_block(tb): by the time this
                    # reaches the SP FIFO head its data is ready, so later
                    # kt/vp prefetches aren't head-of-line blocked behind it
                    nc.sync.dma_start(
                        out_r[:, :, tb * 256:(tb + 1) * 256], o_bigs[tb][:]
                    )

                def load_cache(b, hh):
                    kt_sb = ktpool.tile([128, CACHE], F8E3, name="kt_sb")
                    nc.sync.dma_start(kt_sb[:], kt_d.ap()[hh, b])
                    v_sb = vpool.tile([128, NQ4, 512], F8E3, name="v_sb")
                    nc.sync.dma_start(
                        v_sb[:], vp_d.ap()[hh, b].rearrange("c k d -> k c d")
                    )
                    return kt_sb, v_sb

                def attention(b, hh, pair, cache=None):
                    t, bb = divmod(b, 4)
                    par = pair % 2
                    kt_sb, v_sb = cache if cache else load_cache(b, hh)
                    kt_tail = qkvT[t][:, 2 + hh, bb * 128:(bb + 1) * 128]
                    v_tail = vnew_sb[:, hh, b, :]

                    qT = qkvT[t][:, hh, bb * 128:(bb + 1) * 128]
                    pT = ppool.tile([128, K], F16)
                    acc = ps_acc[:, par * 128:(par + 1) * 128]
                    fold = foldpool.tile([128, 1024], F16)
                    for g in range(4):
                        ps = psum_s.tile([128, 1024], F32, tag="ps_s")
                        for j in range(8):
                            kc = g * 8 + j
                            if kc < NKC - 1:
                                nc.tensor.matmul(
                                    ps[:, j * 128:(j + 1) * 128],
                                    kt_sb[:, kc * 128:(kc + 1) * 128], qT,
                                    start=True, stop=True,
                                )
                            else:
                                # causal mask pre-loaded into PSUM via a
                                # constant matmul (keeps DVE off this path)
                                nc.tensor.matmul(
                                    ps[:, j * 128:(j + 1) * 128],
                                    masktri[:], ident[:],
                                    start=True, stop=False,
                                )
                                nc.tensor.matmul(
                                    ps[:, j * 128:(j + 1) * 128],
                                    kt_tail, qT,
                                    start=False, stop=True,
                                )
                        pslab = pT[:, g * 1024:(g + 1) * 1024]
                        nc.scalar.activation(
                            pslab, ps[:],
                            mybir.ActivationFunctionType.Exp, scale=SCALE,
                        )
                        for j in range(8):
                            c = g * 8 + j
                            c4, jj = divmod(c, 4)
                            lhs = (v_sb[:, c4, jj * 128:(jj + 1) * 128]
                                   if c < NKC - 1 else v_tail)
                            nc.tensor.matmul(
                                acc, lhs, pT[:, c * 128:(c + 1) * 128],
                                start=(c == 0), stop=(c == NKC - 1),
                            )
                        # incremental denominator fold per exp group (keeps
                        # the post-attention serial tail short)
                        if g == 1:
                            nc.vector.tensor_add(
                                fold[:], pT[:, 0:1024], pT[:, 1024:2048]
                            )
                        elif g >= 2:
                            nc.vector.tensor_add(
                                fold[:], fold[:], pT[:, g * 1024:(g + 1) * 1024]
                            )
                    for w in (512, 256, 128):
                        nc.gpsimd.tensor_add(
                            fold[:, 0:w], fold[:, 0:w], fold[:, w:2 * w]
                        )
                    den = ps_acc[0:1, 256 + par * 128:256 + (par + 1) * 128]
                    nc.tensor.matmul(
                        den, ones_col[:], fold[:, 0:128], start=True, stop=True
                    )
                    inv_d = smallpool.tile([1, 128], F32, tag="inv")
                    nc.vector.reciprocal(inv_d[:], den)
                    bcast = smallpool.tile([128, 128], F32, tag="bc")
                    nc.gpsimd.partition_broadcast(bcast[:], inv_d[:])
                    nc.vector.tensor_mul(
                        attn_sb[:, hh, b * 128:(b + 1) * 128], acc, bcast[:]
                    )

                # startup: q for batches 0-1 first; pair-0 cache DMAs queue
                # ahead of the DoubleRow inputs (whose outputs are only
                # needed at the chunk-31 tail of each pair)
                cache00 = load_cache(0, 0)
                q_part(0, 0, 0)
                load_xt8(0)
                nc.sync.dma_start(wq8_sb[:], wq8_d.ap())
                kv_dr(0, 0)        # k0
                kv_dr(0, 2)        # v0
                vnew_transposes(0, 0)
                attention(0, 0, 0, cache00)

                # Non-attention work is spread roughly one piece per pair so
                # the PE never sees a lump; input loads are spread across
                # early batches so the pair kt/vp prefetch stream isn't
                # wedged behind them in the DMA FIFO; output stores all run
                # after the last prefetch.
                def between(pair):
                    if pair == 1:
                        q_part(0, 1, 0)
                        kv_dr(0, 1)
                        kv_dr(0, 3)
                        vnew_transposes(0, 1)
                    elif pair == 2:
                        load_xt(0, 1)
                        q_part(0, 0, 1)
                    elif pair == 3:
                        q_part(0, 1, 1)
                    elif pair == 4:
                        load_xt(1, 0)
                        load_xt8(1)
                    elif pair == 5:
                        kv_dr(1, 0)
                    elif pair == 6:
                        kv_dr(1, 2)
                        vnew_transposes(1, 0)
                        load_xt(1, 1)
                    elif pair == 7:
                        q_part(1, 0, 0)
                        kv_dr(1, 1)
                    elif pair == 8:
                        kv_dr(1, 3)
                        vnew_transposes(1, 1)
                        q_part(1, 1, 0)
                        nc.sync.dma_start(
                            wp_sb[:], wp_d.ap().rearrange("(t p) c -> p t c", p=128)
                        )
                    elif pair == 9:
                        q_part(1, 0, 1)
                        proj_block(0)
                    elif pair == 10:
                        q_part(1, 1, 1)
                    elif pair == 11:
                        proj_block(1)
                    elif pair == 12:
                        proj_store(0)
                    elif pair == 13:
                        proj_block(2)
                    elif pair == 14:
                        proj_store(1)

                for pair in range(1, 16):
                    b, hh = divmod(pair, 2)
                    between(pair)
                    attention(b, hh, pair)
                proj_store(2)
                proj_block(3)
                proj_store(3)

    nc.compile()
    return nc


def prepare_in_maps(x, k_cache, v_cache, Wqkv, Wproj):
    import ml_dtypes

    E3 = np.dtype(ml_dtypes.float8_e3m4)
    E4 = np.dtype(ml_dtypes.float8_e4m3)

    x2 = np.asarray(x, np.float32).reshape(TOK, D)
    xt = np.ascontiguousarray(x2.T, dtype=np.float16)
    # xt8[p, j, i, n] = x^T[(2j+i)*128+p, n]
    xt8 = np.ascontiguousarray(
        np.clip(x2.T, -200, 200).reshape(8, 2, 128, TOK).transpose(2, 0, 1, 3)
    ).astype(E4)

    in_maps = []
    for c in range(NCORES):
        h0 = c * HPC
        # q columns (heads h0, h0+1): [128, 16, 256]
        wqq = np.ascontiguousarray(
            np.asarray(Wqkv[:, h0 * HD:(h0 + HPC) * HD], dtype=np.float16)
            .reshape(16, 128, 256).transpose(1, 0, 2)
        )
        # k/v columns: [k0|k1|v0|v1] -> [128, 8, 2, 512] fp8e4
        kvcols = np.concatenate(
            [Wqkv[:, D + h0 * HD:D + (h0 + HPC) * HD],
             Wqkv[:, 2 * D + h0 * HD:2 * D + (h0 + HPC) * HD]], axis=1
        )  # [2048, 512]
        wq8 = np.ascontiguousarray(
            np.clip(kvcols, -200, 200).reshape(8, 2, 128, 512).transpose(2, 0, 1, 3)
        ).astype(E4)

        ks = k_cache[:, h0:h0 + HPC]                  # [B, HPC, CACHE, HD]
        kt8 = np.ascontiguousarray(
            np.clip(np.transpose(ks, (1, 0, 3, 2)), -14, 14)
        ).astype(E3)                                  # [HPC, B, HD, CACHE]

        vs = np.clip(v_cache[:, h0:h0 + HPC], -14, 14)  # [B, HPC, CACHE, HD]
        vp8 = np.zeros((HPC, B, NQ4, 128, 512), E3)
        full = vs[:, :, :7 * 512, :].reshape(B, HPC, 7, 4, 128, HD)
        vp8[:, :, :7] = np.ascontiguousarray(
            np.transpose(full, (1, 0, 2, 4, 3, 5)).reshape(HPC, B, 7, 128, 512)
        ).astype(E3)
        rem = vs[:, :, 7 * 512:, :].reshape(B, HPC, 3, 128, HD)
        vp8[:, :, 7, :, 0:384] = np.ascontiguousarray(
            np.transpose(rem, (1, 0, 3, 2, 4)).reshape(HPC, B, 128, 384)
        ).astype(E3)

        wp = np.ascontiguousarray(
            Wproj[h0 * HD:(h0 + HPC) * HD, :], dtype=np.float16
        )
        in_maps.append({
            "xt": xt, "xt8": xt8, "wqq": wqq, "wq8": wq8,
            "kt8": kt8, "vp8": vp8, "wp": wp,
        })
    return in_maps


def postprocess(results, bproj):
    total = np.zeros((D, TOK), dtype=np.float32)
    for c in range(NCORES):
        total += results[c]["out"].astype(np.float32)
    out = total.T + bproj.astype(np.float32)[None, :]
    return np.ascontiguousarray(out.reshape(B, Q, D), dtype=np.float32)


def kernel(x, k_cache, v_cache, Wqkv, Wproj, bproj):
    if "nc" not in _STATE:
        _STATE["nc"] = build_nc()
    nc = _STATE["nc"]
    in_maps = prepare_in_maps(
        np.asarray(x), np.asarray(k_cache), np.asarray(v_cache),
        np.asarray(Wqkv), np.asarray(Wproj)
    )
    res = run_bass_kernel_spmd(nc, in_maps, list(range(NCORES)))
    return postprocess(res.results, np.asarray(bproj))


# revision 27
# speedup vs baseline: 54.5585x; 54.5585x over previous
"""Cached multi-head attention, sharded over heads across 8 TRN2 NeuronCores.

Per-core work (2 of 16 heads, all 8 batches), fp8-accelerated:
  - K/V caches stored in fp8e3 (E3M4), halving cache DMA; V quad-packed so
    DMA runs are 512B. The 128 new keys/values are read directly from the
    fp16 qkvT / vnew tiles as the chunk-31 matmul stationaries (never
    quantized to fp8).
  - Scores / attn@V matmuls: fp8e3 stationary x fp16 moving (mixed dtype,
    bf16-rate; ~1.3% rms noise on cache reads only).
  - QKV projection: q columns fp16; k/v columns via fp8e4 DoubleRow
    (256-deep contraction, 2x PE rate; errors land only on the new keys).
  - exp on ACT; causal mask on the last key block; softmax denominator via
    DVE log2 fold + ones-stationary matmul; normalization by reciprocal
    partition-broadcast.
  - PSUM: 2 rotating 2-bank score tiles + a 2-slot QKV pool; one persistent
    bank packs both in-flight attn@V accumulators plus the denominators
    (subtile slices); one persistent bank double-buffers the projection.
  - PSUM->SBUF copies split across ACT/DVE (GpSimd cannot read PSUM);
    the softmax fold tree runs on the idle GpSimd engine.
  - Output projection in 256-token blocks -> one [128,16,256] staging tile
    and a single DMA per block, stores scheduled behind the cache
    prefetch stream; host sums the 8 partials + bias.
"""

import numpy as np

import concourse.bacc as bacc
import concourse.mybir as mybir
import concourse.tile as tile
from concourse.bass_utils import run_bass_kernel_spmd

B, Q, D = 8, 128, 2048
H, HD = 16, 128
CACHE = 3968
K = CACHE + Q          # 4096
NCORES = 8
HPC = H // NCORES      # heads per core
NKC = K // 128         # 32 key chunks (31 cache + 1 new)
NQ4 = 8                # quads of 512 keys for the packed V cache
TOK = B * Q            # 1024 tokens
SCALE = 1.0 / float(np.sqrt(HD))

F16 = mybir.dt.float16
F32 = mybir.dt.float32
F8E3 = mybir.dt.float8e3
F8E4 = mybir.dt.float8e4
DR = mybir.MatmulPerfMode.DoubleRow

_STATE = {}


def build_nc(reps=1):
    nc = bacc.Bacc("TRN2", target_bir_lowering=False, debug=False)

    xt_d = nc.dram_tensor("xt", [D, TOK], F16, kind="ExternalInput")
    xt8_d = nc.dram_tensor("xt8", [128, 8, 2, TOK], F8E4, kind="ExternalInput")
    wqq_d = nc.dram_tensor("wqq", [128, 16, 256], F16, kind="ExternalInput")
    wq8_d = nc.dram_tensor("wq8", [128, 8, 2, 512], F8E4, kind="ExternalInput")
    kt_d = nc.dram_tensor("kt8", [HPC, B, HD, CACHE], F8E3, kind="ExternalInput")
    vp_d = nc.dram_tensor("vp8", [HPC, B, NQ4, 128, 512], F8E3, kind="ExternalInput")
    wp_d = nc.dram_tensor("wp", [HPC * HD, D], F16, kind="ExternalInput")
    out_d = nc.dram_tensor("out", [D, TOK], F16, kind="ExternalOutput")

    with tile.TileContext(nc) as tc:
        with (
            tc.tile_pool(name="const", bufs=1) as cpool,
            tc.tile_pool(name="xw", bufs=1) as xwpool,
            tc.tile_pool(name="qkv", bufs=1) as qkvpool,
            tc.tile_pool(name="vnew", bufs=1) as vnewpool,
            tc.tile_pool(name="attn", bufs=1) as attnpool,
            tc.tile_pool(name="kt", bufs=6) as ktpool,
            tc.tile_pool(name="v", bufs=6) as vpool,
            tc.tile_pool(name="p", bufs=2) as ppool,
            tc.tile_pool(name="fold", bufs=2) as foldpool,
            tc.tile_pool(name="small", bufs=2) as smallpool,
            tc.tile_pool(name="ostage", bufs=4) as opool,
            tc.tile_pool(name="ps_s", bufs=2, space="PSUM") as psum_s,
            tc.tile_pool(name="ps_q", bufs=2, space="PSUM") as psum_q,
            tc.tile_pool(name="ps_acc", bufs=1, space="PSUM") as psum_a,
            tc.tile_pool(name="ps_proj", bufs=1, space="PSUM") as psum_p,
        ):
            # constants
            ones_col = cpool.tile([128, 1], F16)       # denominator stationary
            nc.vector.memset(ones_col[:], 1.0)
            ones_full = cpool.tile([128, 128], F16)
            nc.vector.memset(ones_full[:], 1.0)
            # causal mask for the last key block: keep (p=key j', free=query i)
            # where i >= j'  -> iota = i - j' >= 0
            zeros16 = cpool.tile([128, 128], F16)
            nc.vector.memset(zeros16[:], 0.0)
            # masktri[p=i, col=j'] = -60000 where j' > i (strict upper):
            # its transpose, accumulated into the chunk-31 score PSUM via a
            # constant matmul against the identity, applies the causal mask
            masktri = cpool.tile([128, 128], F16)
            nc.gpsimd.affine_select(
                masktri[:], zeros16[:], pattern=[[-1, 128]],
                compare_op=mybir.AluOpType.is_ge, fill=-60000.0,
                base=0, channel_multiplier=1,
            )
            ident = cpool.tile([128, 128], F16)
            nc.gpsimd.affine_select(
                ident[:], ones_full[:], pattern=[[1, 128]],
                compare_op=mybir.AluOpType.is_equal, fill=0.0,
                base=0, channel_multiplier=-1,
            )

            for _rep in range(reps):
                # one persistent bank: attn@V accumulators for both in-flight
                # pairs ([:,0:128],[:,128:256]) + denominators on partition 0
                # ([0:1,256:384],[0:1,384:512])
                ps_acc = psum_a.tile([128, 512], F32, tag="acc", name="ps_acc")
                # one persistent bank double-buffering the projection
                ps_proj = psum_p.tile([128, 512], F32, tag="proj", name="ps_proj")

                xt8_half = [None, None]
                xt_half = [None, None]

                def load_xt8(t):
                    xt8_half[t] = xwpool.tile([128, 8, 2, 512], F8E4,
                                              tag="xt8", name=f"xt8_{t}")
                    nc.sync.dma_start(
                        xt8_half[t][:], xt8_d.ap()[:, :, :, t * 512:(t + 1) * 512]
                    )

                def load_xt(t, s):
                    if xt_half[t] is None:
                        xt_half[t] = xwpool.tile([128, D // 128, 512], F16,
                                                 tag="xt", name=f"xt{t}")
                    nc.sync.dma_start(
                        xt_half[t][:, :, s * 256:(s + 1) * 256],
                        xt_d.ap().rearrange("(t p) n -> p t n", p=128)
                        [:, :, t * 512 + s * 256:t * 512 + (s + 1) * 256],
                    )

                wq8_sb = xwpool.tile([128, 8, 2, 512], F8E4, tag="wq8", name="wq8_sb")
                wqq_sb = xwpool.tile([128, 16, 256], F16, tag="wqq", name="wqq_sb")

                nc.sync.dma_start(wqq_sb[:], wqq_d.ap())
                load_xt(0, 0)

                # qkvT oc layout: [q0, q1, k0, k1, v0, v1] each [128, 512]
                qkvT = [
                    qkvpool.tile([128, 6, 512], F16, tag=f"qkvT{t}", name=f"qkvT{t}")
                    for t in range(2)
                ]
                vnew_sb = vnewpool.tile([128, HPC, B, HD], F16, tag="vnew",
                                        name="vnew_sb")

                def kv_dr(t, cb):
                    # k/v projection columns via fp8e4 DoubleRow (256-contraction)
                    ps = psum_q.tile([128, 512], F32, tag="ps_q", name="ps_kv")
                    for j in range(8):
                        nc.tensor.matmul(
                            ps[:],
                            wq8_sb[:, j, :, cb * 128:(cb + 1) * 128],
                            xt8_half[t][:, j, :, :],
                            start=(j == 0), stop=(j == 7),
                            perf_mode=DR,
                        )
                    nc.scalar.copy(qkvT[t][:, 2 + cb, :], ps[:])

                def q_part(t, hh, s):
                    # q columns, fp16, one 256-token slab
                    ps = psum_q.tile([128, 512], F32, tag="ps_q", name="ps_q")[:, 0:256]
                    for dt_ in range(D // 128):
                        nc.tensor.matmul(
                            ps[:],
                            wqq_sb[:, dt_, hh * 128:(hh + 1) * 128],
                            xt_half[t][:, dt_, s * 256:(s + 1) * 256],
                            start=(dt_ == 0), stop=(dt_ == D // 128 - 1),
                        )
                    nc.scalar.copy(
                        qkvT[t][:, hh, s * 256:(s + 1) * 256], ps[:]
                    )

                def vnew_transposes(t, hh):
                    # v_new natural layout via PE transpose for this half's batches
                    for bb in range(4):
                        b = 4 * t + bb
                        ps_t = psum_q.tile([128, 512], F32, tag="ps_q",
                                           name="ps_vt").bitcast(mybir.dt.float16)[:, 0:128]
                        nc.tensor.transpose(
                            ps_t[:], qkvT[t][:, 4 + hh, bb * 128:(bb + 1) * 128],
                            ident[:],
                        )
                        nc.vector.tensor_copy(vnew_sb[:, hh, b, :], ps_t[:])

                attn_sb = attnpool.tile([128, HPC, TOK], F16, tag="attn",
                                        name="attn_sb")
                wp_sb = xwpool.tile([128, HPC, D], F16, tag="wp", name="wp_sb")
                out_r = out_d.ap().rearrange("(cc p) n -> p cc n", p=128)

                o_bigs = {}

                def proj_block(tb):
                    # partial final^T for 256 tokens (batches 2tb, 2tb+1)
                    o_big = opool.tile([128, 16, 256], F16)
                    o_bigs[tb] = o_big
                    for cc in range(D // 128):
                        psl = ps_proj[:, (cc % 2) * 256:(cc % 2) * 256 + 256]
                        for ht in range(HPC):
                            nc.tensor.matmul(
                                psl,
                                wp_sb[:, ht, cc * 128:(cc + 1) * 128],
                                attn_sb[:, ht, tb * 256:(tb + 1) * 256],
                                start=(ht == 0), stop=(ht == HPC - 1),
                            )
                        if cc % 8 == 1:
                            nc.scalar.copy(o_big[:, cc, :], psl)
                        else:
                            nc.vector.tensor_copy(o_big[:, cc, :], psl)
                def proj_store(tb):
                    # emitted one batch after proj_block(tb): by the time this
                    # reaches the SP FIFO head its data is ready, so later
                    # kt/vp prefetches aren't head-of-line blocked behind it
                    nc.sync.dma_start(
                        out_r[:, :, tb * 256:(tb + 1) * 256], o_bigs[tb][:]
                    )

                def load_cache(b, hh):
                    kt_sb = ktpool.tile([128, CACHE], F8E3, name="kt_sb")
                    nc.sync.dma_start(kt_sb[:], kt_d.ap()[hh, b])
                    v_sb = vpool.tile([128, NQ4, 512], F8E3, name="v_sb")
                    nc.sync.dma_start(
                        v_sb[:], vp_d.ap()[hh, b].rearrange("c k d -> k c d")
                    )
                    return kt_sb, v_sb

                def attention(b, hh, pair, cache=None):
                    t, bb = divmod(b, 4)
                    par = pair % 2
                    kt_sb, v_sb = cache if cache else load_cache(b, hh)
                    kt_tail = qkvT[t][:, 2 + hh, bb * 128:(bb + 1) * 128]
                    v_tail = vnew_sb[:, hh, b, :]

                    qT = qkvT[t][:, hh, bb * 128:(bb + 1) * 128]
                    pT = ppool.tile([128, K], F16)
                    acc = ps_acc[:, par * 128:(par + 1) * 128]
                    fold = foldpool.tile([128, 1024], F16)
                    for g in range(4):
                        ps = psum_s.tile([128, 1024], F32, tag="ps_s")
                        for j in range(8):
                            kc = g * 8 + j
                            if kc < NKC - 1:
                                nc.tensor.matmul(
                                    ps[:, j * 128:(j + 1) * 128],
                                    kt_sb[:, kc * 128:(kc + 1) * 128], qT,
                                    start=True, stop=True,
                                )
                            else:
                                # causal mask pre-loaded into PSUM via a
                                # constant matmul (keeps DVE off this path)
                                nc.tensor.matmul(
                                    ps[:, j * 128:(j + 1) * 128],
                                    masktri[:], ident[:],
                                    start=True, stop=False,
                                )
                                nc.tensor.matmul(
                                    ps[:, j * 128:(j + 1) * 128],
                                    kt_tail, qT,
                                    start=False, stop=True,
                                )
                        pslab = pT[:, g * 1024:(g + 1) * 1024]
                        nc.scalar.activation(
                            pslab, ps[:],
                            mybir.ActivationFunctionType.Exp, scale=SCALE,
                        )
                        for j in range(8):
                            c = g * 8 + j
                            c4, jj = divmod(c, 4)
                            lhs = (v_sb[:, c4, jj * 128:(jj + 1) * 128]
                                   if c < NKC - 1 else v_tail)
                            nc.tensor.matmul(
                                acc, lhs, pT[:, c * 128:(c + 1) * 128],
                                start=(c == 0), stop=(c == NKC - 1),
                            )
                        # incremental denominator fold per exp group (keeps
                        # the post-attention serial tail short)
                        if g == 1:
                            nc.vector.tensor_add(
                                fold[:], pT[:, 0:1024], pT[:, 1024:2048]
                            )
                        elif g >= 2:
                            nc.vector.tensor_add(
                                fold[:], fold[:], pT[:, g * 1024:(g + 1) * 1024]
                            )
                    for w in (512, 256, 128):
                        nc.gpsimd.tensor_add(
                            fold[:, 0:w], fold[:, 0:w], fold[:, w:2 * w]
                        )
                    den = ps_acc[0:1, 256 + par * 128:256 + (par + 1) * 128]
                    nc.tensor.matmul(
                        den, ones_col[:], fold[:, 0:128], start=True, stop=True
                    )
                    inv_d = smallpool.tile([1, 128], F32, tag="inv")
                    nc.vector.reciprocal(inv_d[:], den)
                    bcast = smallpool.tile([128, 128], F32, tag="bc")
                    nc.gpsimd.partition_broadcast(bcast[:], inv_d[:])
                    nc.vector.tensor_mul(
                        attn_sb[:, hh, b * 128:(b + 1) * 128], acc, bcast[:]
                    )

                # startup: q for batches 0-1 first; pair-0 cache DMAs queue
                # ahead of the DoubleRow inputs (whose outputs are only
                # needed at the chunk-31 tail of each pair)
                cache00 = load_cache(0, 0)
                q_part(0, 0, 0)
                load_xt8(0)
                nc.sync.dma_start(wq8_sb[:], wq8_d.ap())
                kv_dr(0, 0)        # k0
                kv_dr(0, 2)        # v0
                vnew_transposes(0, 0)
                attention(0, 0, 0, cache00)

                # Non-attention work is spread roughly one piece per pair so
                # the PE never sees a lump; input loads are spread across
                # early batches so the pair kt/vp prefetch stream isn't
                # wedged behind them in the DMA FIFO; output stores all run
                # after the last prefetch.
                def between(pair):
                    if pair == 1:
                        q_part(0, 1, 0)
                        kv_dr(0, 1)
                        kv_dr(0, 3)
                        vnew_transposes(0, 1)
                    elif pair == 2:
                        load_xt(0, 1)
                        q_part(0, 0, 1)
                    elif pair == 3:
                        q_part(0, 1, 1)
                    elif pair == 4:
                        load_xt(1, 0)
                        load_xt8(1)
                    elif pair == 5:
                        kv_dr(1, 0)
                    elif pair == 6:
                        kv_dr(1, 2)
                        vnew_transposes(1, 0)
                        load_xt(1, 1)
                    elif pair == 7:
                        q_part(1, 0, 0)
                        kv_dr(1, 1)
                    elif pair == 8:
                        kv_dr(1, 3)
                        vnew_transposes(1, 1)
                        q_part(1, 1, 0)
                        nc.sync.dma_start(
                            wp_sb[:], wp_d.ap().rearrange("(t p) c -> p t c", p=128)
                        )
                    elif pair == 9:
                        q_part(1, 0, 1)
                        proj_block(0)
                    elif pair == 10:
                        q_part(1, 1, 1)
                    elif pair == 11:
                        proj_block(1)
                    elif pair == 12:
                        proj_store(0)
                    elif pair == 13:
                        proj_block(2)
                    elif pair == 14:
                        proj_store(1)

                for pair in range(1, 16):
                    b, hh = divmod(pair, 2)
                    between(pair)
                    attention(b, hh, pair)
                proj_store(2)
                proj_block(3)
                proj_store(3)

    nc.compile()
    return nc


def prepare_in_maps(x, k_cache, v_cache, Wqkv, Wproj):
    import ml_dtypes

    E3 = np.dtype(ml_dtypes.float8_e3m4)
    E4 = np.dtype(ml_dtypes.float8_e4m3)

    x2 = np.asarray(x, np.float32).reshape(TOK, D)
    xt = np.ascontiguousarray(x2.T, dtype=np.float16)
    # xt8[p, j, i, n] = x^T[(2j+i)*128+p, n]
    xt8 = np.ascontiguousarray(
        np.clip(x2.T, -200, 200).reshape(8, 2, 128, TOK).transpose(2, 0, 1, 3)
    ).astype(E4)

    in_maps = []
    for c in range(NCORES):
        h0 = c * HPC
        # q columns (heads h0, h0+1): [128, 16, 256]
        wqq = np.ascontiguousarray(
            np.asarray(Wqkv[:, h0 * HD:(h0 + HPC) * HD], dtype=np.float16)
            .reshape(16, 128, 256).transpose(1, 0, 2)
        )
        # k/v columns: [k0|k1|v0|v1] -> [128, 8, 2, 512] fp8e4
        kvcols = np.concatenate(
            [Wqkv[:, D + h0 * HD:D + (h0 + HPC) * HD],
             Wqkv[:, 2 * D + h0 * HD:2 * D + (h0 + HPC) * HD]], axis=1
        )  # [2048, 512]
        wq8 = np.ascontiguousarray(
            np.clip(kvcols, -200, 200).reshape(8, 2, 128, 512).transpose(2, 0, 1, 3)
        ).astype(E4)

        ks = k_cache[:, h0:h0 + HPC]                  # [B, HPC, CACHE, HD]
        kt8 = np.ascontiguousarray(
            np.clip(np.transpose(ks, (1, 0, 3, 2)), -14, 14)
        ).astype(E3)                                  # [HPC, B, HD, CACHE]

        vs = np.clip(v_cache[:, h0:h0 + HPC], -14, 14)  # [B, HPC, CACHE, HD]
        vp8 = np.zeros((HPC, B, NQ4, 128, 512), E3)
        full = vs[:, :, :7 * 512, :].reshape(B, HPC, 7, 4, 128, HD)
        vp8[:, :, :7] = np.ascontiguousarray(
            np.transpose(full, (1, 0, 2, 4, 3, 5)).reshape(HPC, B, 7, 128, 512)
        ).astype(E3)
        rem = vs[:, :, 7 * 512:, :].reshape(B, HPC, 3, 128, HD)
        vp8[:, :, 7, :, 0:384] = np.ascontiguousarray(
            np.transpose(rem, (1, 0, 3, 2, 4)).reshape(HPC, B, 128, 384)
        ).astype(E3)

        wp = np.ascontiguousarray(
            Wproj[h0 * HD:(h0 + HPC) * HD, :], dtype=np.float16
        )
        in_maps.append({
            "xt": xt, "xt8": xt8, "wqq": wqq, "wq8": wq8,
            "kt8": kt8, "vp8": vp8, "wp": wp,
        })
    return in_maps


def postprocess(results, bproj):
    total = np.zeros((D, TOK), dtype=np.float32)
    for c in range(NCORES):
        total += results[c]["out"].astype(np.float32)
    out = total.T + bproj.astype(np.float32)[None, :]
    return np.ascontiguousarray(out.reshape(B, Q, D), dtype=np.float32)


def kernel(x, k_cache, v_cache, Wqkv, Wproj, bproj):
    if "nc" not in _STATE:
        _STATE["nc"] = build_nc()
    nc = _STATE["nc"]
    in_maps = prepare_in_maps(
        np.asarray(x), np.asarray(k_cache), np.asarray(v_cache),
        np.asarray(Wqkv), np.asarray(Wproj)
    )
    res = run_bass_kernel_spmd(nc, in_maps, list(range(NCORES)))
    return postprocess(res.results, np.asarray(bproj))
